# revision 7
# baseline (speedup 1.0000x reference)
"""OGRENet GNN message-passing kernel for 8 Trainium2 NeuronCores.

Strategy
--------
Host (numpy, cheap index plumbing only):
  * u2 = u @ Wsel + bsel  (64x256, negligible FLOPs)
  * sort edges by destination node (`row`), split into 8 contiguous chunks at
    node boundaries (2500 nodes per core) -> each core owns a contiguous node
    range and ALL edges that scatter into it => no cross-core reduction.
  * pack per-core feature-major edge inputs ein.T = [x[col]; x[row]; e_attr;
    u2[batch[row]]] (fp16), per-edge 1/count scales, window-relative row ids.

Device (per core, identical program, different data):
  * edge MLP (275->1024x4->512) + node MLP1 (521->512->512) as fp16 matmuls,
    feature-major activations [feat_part, edge_free], fp32 PSUM accumulation.
  * node MLP1 layer2 flips to edge-major [128 edges, 512 feat] so the
    segment-sum becomes a matmul: membership tile M[e, n] = (row[e]==n)
    (built on DVE via is_equal against an iota row) contracts edges away:
    agg.T[f, n] += h.T_tile @ M.  1/count is folded into h via the ACT scale.
  * PSUM accumulates each 256-node window over a static range of edge tiles
    (ranges computed from the actual data on host, shared by all cores;
    membership zeroes any edge outside the window, so overlap is harmless).
  * node MLP2 (777->512->1) consumes agg feature-major directly; z -> DRAM.
"""

import os
import sys

import numpy as np

sys.path.insert(0, "/opt/trn_rl_repo")

N_NODES = 20000
N_GRAPHS = 64
U_DIM = 256
E_HID = 1024
E_OUT = 512
N_HID = 512
NC = 8
NPN = N_NODES // NC          # nodes per core (2500)
NPAD = 2560                  # padded nodes per core
WN = 256                     # nodes per segment window
NWIN = NPAD // WN            # 10
P = 128

_CACHE = {}


def _pack_cols(v, T):
    """[T*128] -> [128, T] with col t = v[t*128:(t+1)*128]."""
    return np.ascontiguousarray(v.reshape(T, P).T)


def _build_module(EPAD, win_tiles, h2_bufs):
    """Build the per-core Bass program. win_tiles[w] = (tlo, thi) inclusive."""
    from concourse import bacc, mybir, tile

    T = EPAD // P           # 128-edge tiles
    NB = EPAD // 1024       # 1024-edge blocks
    f16 = mybir.dt.float16
    f32 = mybir.dt.float32
    RELU = mybir.ActivationFunctionType.Relu
    COPY = mybir.ActivationFunctionType.Copy
    IDENT = mybir.ActivationFunctionType.Identity

    nc = bacc.Bacc(None, target_bir_lowering=False, debug=False)

    with tile.TileContext(nc) as tc:
        with (
            tc.tile_pool(name="dram", bufs=1, space="DRAM") as dram,
            tc.tile_pool(name="wres", bufs=1) as wres,
            tc.tile_pool(name="einp", bufs=5) as einp,
            tc.tile_pool(name="actp", bufs=8) as actp,
            tc.tile_pool(name="act4", bufs=4) as act4,
            tc.tile_pool(name="h2p", bufs=h2_bufs) as h2p,
            tc.tile_pool(name="smal", bufs=3) as smal,
            tc.tile_pool(name="aggp", bufs=8) as aggp,
            tc.tile_pool(name="strm", bufs=4) as strm,
            tc.tile_pool(name="bigps", bufs=3, space="PSUM") as bigps,
            tc.tile_pool(name="segps", bufs=4, space="PSUM") as segps,
        ):
            # ---- DRAM I/O -------------------------------------------------
            d_ein = dram.tile([384, EPAD], f16, kind="ExternalInput", name="ein")
            d_relw = dram.tile([P, NWIN * T], f32, kind="ExternalInput", name="relw")
            d_invc = dram.tile([P, T], f32, kind="ExternalInput", name="invc")
            d_xT2 = dram.tile([P, NPAD], f16, kind="ExternalInput", name="xT2")
            d_u2bT = dram.tile([U_DIM, NPAD], f16, kind="ExternalInput", name="u2bT")
            d_iota = dram.tile([P, WN], f32, kind="ExternalInput", name="iota")
            d_n1b2bc = dram.tile([P, 512], f32, kind="ExternalInput", name="n1b2bc")

            wspec = dict(
                eW1p=[384, E_HID], eW2=[E_HID, E_HID], eW3=[E_HID, E_HID],
                eW4=[E_HID, E_HID], eW5=[E_HID, E_OUT],
                n1W1a=[P, N_HID], n1W1b=[E_OUT, N_HID], n1W2=[N_HID, N_HID],
                n2W1x=[P, N_HID], n2W1agg=[N_HID, N_HID], n2W1u=[U_DIM, N_HID],
                n2W2=[N_HID, 1],
            )
            d_w = {k: dram.tile(s, f16, kind="ExternalInput", name=k)
                   for k, s in wspec.items()}
            bspec = dict(eb1r=[P, 8], eb2r=[P, 8], eb3r=[P, 8], eb4r=[P, 8],
                         eb5r=[P, 4], n1b1r=[P, 4], n2b1r=[P, 4], n2b2r=[1, 1])
            d_b = {k: dram.tile(s, f32, kind="ExternalInput", name=k)
                   for k, s in bspec.items()}
            d_z = dram.tile([1, NPAD], f32, kind="ExternalOutput", name="zout")

            names = dict(ein=d_ein.name, relw=d_relw.name, invc=d_invc.name,
                         xT2=d_xT2.name, u2bT=d_u2bT.name, iota=d_iota.name,
                         n1b2bc=d_n1b2bc.name, zout=d_z.name)
            names.update({k: v.name for k, v in d_w.items()})
            names.update({k: v.name for k, v in d_b.items()})

            # ---- resident loads ------------------------------------------
            def load_w(name, nk, width):
                ts = []
                for k in range(nk):
                    t = wres.tile([P, width], f16, name=f"w_{name}_{k}")
                    nc.sync.dma_start(out=t[:], in_=d_w[name][k * P:(k + 1) * P, :])
                    ts.append(t)
                return ts

            W1 = load_w("eW1p", 3, E_HID)
            W2 = load_w("eW2", 8, E_HID)
            W3 = load_w("eW3", 8, E_HID)
            W4 = load_w("eW4", 8, E_HID)
            W5 = load_w("eW5", 8, E_OUT)
            W6a = load_w("n1W1a", 1, N_HID)
            W6b = load_w("n1W1b", 4, N_HID)
            W7 = load_w("n1W2", 4, N_HID)
            W8x = load_w("n2W1x", 1, N_HID)
            W8a = load_w("n2W1agg", 4, N_HID)
            W8u = load_w("n2W1u", 2, N_HID)
            W9 = load_w("n2W2", 4, 1)

            B = {}
            for k, s in bspec.items():
                t = wres.tile(s, f32, name=f"b_{k}")
                nc.sync.dma_start(out=t[:], in_=d_b[k][:])
                B[k] = t
            relw = wres.tile([P, NWIN * T], f32, name="relw_sb")
            nc.sync.dma_start(out=relw[:], in_=d_relw[:])
            invc = wres.tile([P, T], f32, name="invc_sb")
            nc.sync.dma_start(out=invc[:], in_=d_invc[:])
            iota = wres.tile([P, WN], f32, name="iota_sb")
            nc.sync.dma_start(out=iota[:], in_=d_iota[:])
            n1b2bc = wres.tile([P, 512], f32, name="n1b2bc_sb")
            nc.sync.dma_start(out=n1b2bc[:], in_=d_n1b2bc[:])
            xT2 = wres.tile([P, NPAD], f16, name="xT2_sb")
            nc.sync.dma_start(out=xT2[:], in_=d_xT2[:])

            # ---- helpers --------------------------------------------------
            def fm_layer(ins, Wt, bias, nM, width, act, out_pool, tag):
                """feature-major layer: out[m] [128, width_edges]"""
                outs = []
                for m in range(nM):
                    ps = bigps.tile([P, 512], mybir.dt.float32, name="ps_big")
                    pss = ps[:, :width]
                    for ki, (it, wt) in enumerate(zip(ins, Wt)):
                        nc.tensor.matmul(
                            out=pss, lhsT=wt[:, m * P:(m + 1) * P], rhs=it,
                            start=(ki == 0), stop=(ki == len(ins) - 1))
                    ot = out_pool.tile([P, width], f16, name=tag)
                    nc.scalar.activation(ot[:], pss, act, bias=bias[:, m:m + 1])
                    outs.append(ot)
                return outs

            # h2 tiles by global tile index
            h2_tiles = {}

            def emit_block(b):
                e0 = b * 1024
                ein = []
                for k in range(3):
                    t = einp.tile([P, 1024], f16, name="ein_t")
                    nc.sync.dma_start(out=t[:], in_=d_ein[k * P:(k + 1) * P, e0:e0 + 1024])
                    ein.append(t)
                for h in range(2):
                    hs = slice(h * 512, (h + 1) * 512)
                    einh = [e[:, hs] for e in ein]
                    a1 = fm_layer(einh, W1, B["eb1r"], 8, 512, RELU, actp, "actA")
                    a2 = fm_layer([a[:] for a in a1], W2, B["eb2r"], 8, 512,
                                  RELU, actp, "actB")
                    a3 = fm_layer([a[:] for a in a2], W3, B["eb3r"], 8, 512,
                                  RELU, actp, "actA")
                    a4 = fm_layer([a[:] for a in a3], W4, B["eb4r"], 8, 512,
                                  RELU, actp, "actB")
                    e5 = fm_layer([a[:] for a in a4], W5, B["eb5r"], 4, 512,
                                  IDENT, act4, "actC")
                    h1 = fm_layer([einh[0]] + [a[:] for a in e5],
                                  [W6a[0]] + W6b, B["n1b1r"], 4, 512, RELU,
                                  act4, "actD")
                    # n1 layer2 -> edge-major h2 per 128-edge subtile, then
                    # bias-add (DVE) + relu*invc (ACT) -> fp16
                    for s in range(4):
                        t_glob = b * 8 + h * 4 + s
                        ps = bigps.tile([P, 512], mybir.dt.float32, name="ps_big")
                        pss = ps[:, :512]
                        for ki in range(4):
                            nc.tensor.matmul(
                                out=pss, lhsT=h1[ki][:, s * P:(s + 1) * P],
                                rhs=W7[ki][:], start=(ki == 0), stop=(ki == 3))
                        tmp = smal.tile([P, 512], f16, name="tmp16")
                        nc.vector.tensor_tensor(out=tmp[:], in0=pss, in1=n1b2bc[:],
                                                op=mybir.AluOpType.add)
                        h2 = h2p.tile([P, 512], f16, name="h2t")
                        nc.scalar.activation(h2[:], tmp[:], RELU,
                                             scale=invc[:, t_glob:t_glob + 1])
                        h2_tiles[t_glob] = h2

            def emit_window(w):
                tlo, thi = win_tiles[w]
                seg = [segps.tile([P, WN], mybir.dt.float32, name="segps_t")
                       for _ in range(4)]
                tl = list(range(tlo, thi + 1))
                for si, t in enumerate(tl):
                    memb = smal.tile([P, WN], f16, name="memb")
                    nc.vector.tensor_scalar(
                        out=memb[:], in0=iota[:],
                        scalar1=relw[:, w * T + t:w * T + t + 1], scalar2=None,
                        op0=mybir.AluOpType.is_equal)
                    h2 = h2_tiles[t]
                    for fc in range(4):
                        nc.tensor.matmul(
                            out=seg[fc][:], lhsT=h2[:, fc * P:(fc + 1) * P],
                            rhs=memb[:], start=(si == 0), stop=(si == len(tl) - 1))
                agg = []
                for fc in range(4):
                    at = aggp.tile([P, WN], f16, name="aggt")
                    nc.scalar.activation(at[:], seg[fc][:], COPY)
                    agg.append(at)
                # ---- node MLP2 on this 256-node window ----
                n0 = w * WN
                u2c = []
                for k in range(2):
                    t = strm.tile([P, WN], f16, name="u2c")
                    nc.sync.dma_start(out=t[:], in_=d_u2bT[k * P:(k + 1) * P,
                                                          n0:n0 + WN])
                    u2c.append(t)
                xin = [xT2[:, n0:n0 + WN]] + [a[:] for a in agg] + \
                      [u2c[0][:], u2c[1][:]]
                Win = [W8x[0]] + W8a + W8u
                z1 = []
                for m in range(4):
                    ps = bigps.tile([P, 512], mybir.dt.float32, name="ps_big")
                    pss = ps[:, :WN]
                    for ki in range(7):
                        nc.tensor.matmul(out=pss, lhsT=Win[ki][:, m * P:(m + 1) * P],
                                         rhs=xin[ki], start=(ki == 0), stop=(ki == 6))
                    zt = smal.tile([P, WN], f16, name="z1t")
                    nc.scalar.activation(zt[:], pss, RELU,
                                         bias=B["n2b1r"][:, m:m + 1])
                    z1.append(zt)
                ps = bigps.tile([P, 512], mybir.dt.float32, name="ps_big")
                pss = ps[:1, :WN]
                for ki in range(4):
                    nc.tensor.matmul(out=pss, lhsT=W9[ki][:], rhs=z1[ki][:],
                                     start=(ki == 0), stop=(ki == 3))
                zo = smal.tile([1, WN], f32, name="zot")
                nc.scalar.activation(zo[:], pss, IDENT, bias=B["n2b2r"][:])
                nc.sync.dma_start(out=d_z[:, n0:n0 + WN], in_=zo[:])

            # window w ready once tile win_tiles[w][1] is produced
            ready = {}
            for w in range(NWIN):
                b_ready = min(NB - 1, win_tiles[w][1] // 8)
                ready.setdefault(b_ready, []).append(w)
            for b in range(NB):
                emit_block(b)
                for w in ready.get(b, []):
                    emit_window(w)

    nc.compile()
    return nc, names


def kernel(x, edge_attr, u, edge_index, batch, Wsel, bsel,
           eW1, eb1, eW2, eb2, eW3, eb3, eW4, eb4, eW5, eb5,
           n1W1, n1b1, n1W2, n1b2, n2W1, n2b1, n2W2, n2b2):
    f32 = np.float32
    f16 = np.float16
    x = np.asarray(x, f32)
    edge_attr = np.asarray(edge_attr, f32)
    u = np.asarray(u, f32)
    edge_index = np.asarray(edge_index)
    batch = np.asarray(batch)
    ws = {k: np.asarray(v, f32) for k, v in dict(
        Wsel=Wsel, bsel=bsel, eW1=eW1, eb1=eb1, eW2=eW2, eb2=eb2, eW3=eW3,
        eb3=eb3, eW4=eW4, eb4=eb4, eW5=eW5, eb5=eb5, n1W1=n1W1, n1b1=n1b1,
        n1W2=n1W2, n1b2=n1b2, n2W1=n2W1, n2b1=n2b1, n2W2=n2W2, n2b2=n2b2).items()}

    # ---------------- host math (index plumbing + tiny matmul) -------------
    u2 = (u @ ws["Wsel"] + ws["bsel"]).astype(f32)          # [64, 256]
    row = np.asarray(edge_index[0], np.int64)
    col = np.asarray(edge_index[1], np.int64)
    order = np.argsort(row, kind="stable")
    row_s, col_s = row[order], col[order]
    g_s = batch[row_s]
    ea_s = edge_attr[order, 0]
    cnt = np.bincount(row, minlength=N_NODES).astype(f32)
    invc_node = (1.0 / np.maximum(cnt, 1.0)).astype(f32)

    bounds = np.searchsorted(row_s, np.arange(0, N_NODES + 1, NPN))
    e_cnt = np.diff(bounds)
    EPAD = int(-(-int(e_cnt.max()) // 1024) * 1024)
    T = EPAD // P

    # per-window tile ranges (shared across cores) for the static program
    tlo = np.full(NWIN, T - 1, np.int64)
    thi = np.zeros(NWIN, np.int64)
    core_dat = []
    for c in range(NC):
        lo, hi = bounds[c], bounds[c + 1]
        n = hi - lo
        rel = np.full(EPAD, 1e6, f32)
        rel[:n] = (row_s[lo:hi] - NPN * c).astype(f32)
        w_of_edge = np.floor_divide(rel[:n].astype(np.int64), WN)
        for w in range(NWIN):
            idx = np.nonzero(w_of_edge == w)[0]
            if idx.size:
                tlo[w] = min(tlo[w], idx[0] // P)
                thi[w] = max(thi[w], idx[-1] // P)
        core_dat.append((lo, hi, n, rel))
    win_tiles = [(int(tlo[w]), int(max(tlo[w], thi[w]))) for w in range(NWIN)]
    h2_bufs = max(hw - lw + 1 for lw, hw in win_tiles) + 14

    # ---------------- per-core input packing --------------------------------
    zpad109 = np.zeros((109, ws["eW1"].shape[1]), f32)
    eW1p = np.concatenate([ws["eW1"][9:18], ws["eW1"][0:9], ws["eW1"][18:19],
                           ws["eW1"][19:275], zpad109], axis=0)
    n1W1a = np.zeros((P, N_HID), f32)
    n1W1a[0:9] = ws["n1W1"][0:9]
    n2W1x = np.zeros((P, N_HID), f32)
    n2W1x[0:9] = ws["n2W1"][0:9]

    def br(b, nm):   # bias [nm*128] -> [128, nm]
        return np.ascontiguousarray(b.reshape(nm, P).T).astype(f32)

    shared = dict(
        eW1p=eW1p.astype(f16), eW2=ws["eW2"].astype(f16),
        eW3=ws["eW3"].astype(f16), eW4=ws["eW4"].astype(f16),
        eW5=ws["eW5"].astype(f16), n1W1a=n1W1a.astype(f16),
        n1W1b=ws["n1W1"][9:521].astype(f16), n1W2=ws["n1W2"].astype(f16),
        n2W1x=n2W1x.astype(f16), n2W1agg=ws["n2W1"][9:521].astype(f16),
        n2W1u=ws["n2W1"][521:777].astype(f16), n2W2=ws["n2W2"].astype(f16),
        eb1r=br(ws["eb1"], 8), eb2r=br(ws["eb2"], 8), eb3r=br(ws["eb3"], 8),
        eb4r=br(ws["eb4"], 8), eb5r=br(ws["eb5"], 4), n1b1r=br(ws["n1b1"], 4),
        n2b1r=br(ws["n2b1"], 4), n2b2r=ws["n2b2"].reshape(1, 1).astype(f32),
        iota=np.tile(np.arange(WN, dtype=f32), (P, 1)),
        n1b2bc=np.tile(ws["n1b2"].astype(f32), (P, 1)),
    )

    in_maps = []
    for c in range(NC):
        lo, hi, n, rel = core_dat[c]
        ein = np.zeros((384, EPAD), f16)
        ein[0:9, :n] = x[col_s[lo:hi]].T
        ein[9:18, :n] = x[row_s[lo:hi]].T
        ein[18, :n] = ea_s[lo:hi]
        ein[19:275, :n] = u2[g_s[lo:hi]].T
        relw = np.empty((P, NWIN * T), f32)
        for w in range(NWIN):
            relw[:, w * T:(w + 1) * T] = _pack_cols(rel - float(WN) * w, T)
        invc_e = np.ones(EPAD, f32)
        invc_e[:n] = invc_node[row_s[lo:hi]]
        xT2 = np.zeros((P, NPAD), f16)
        xT2[0:9, :NPN] = x[NPN * c:NPN * (c + 1)].T
        u2bT = np.zeros((U_DIM, NPAD), f16)
        u2bT[:, :NPN] = u2[batch[NPN * c:NPN * (c + 1)]].T
        im = dict(shared)
        im.update(ein=ein, relw=relw, invc=_pack_cols(invc_e, T),
                  xT2=xT2, u2bT=u2bT)
        in_maps.append(im)

    # ---------------- build + run ------------------------------------------
    key = (EPAD, tuple(win_tiles))
    if key not in _CACHE:
        _CACHE[key] = _build_module(EPAD, win_tiles, h2_bufs)
    nc, names = _CACHE[key]

    from concourse import bass_utils
    trace = bool(int(os.environ.get("KERNEL_TRACE", "0")))
    if trace:
        try:
            import types
            import antenv
            if not hasattr(antenv, "axon_hooks"):
                mod = types.ModuleType("antenv.axon_hooks")
                mod._hook = None
                mod.set_axon_ntff_profile_hook = lambda h: setattr(mod, "_hook", h)
                mod.get_axon_ntff_profile_hook = lambda: mod._hook
                sys.modules["antenv.axon_hooks"] = mod
                antenv.axon_hooks = mod
                from trn_agent_boot.trn_boot import _ntff_profile_via_ctypes
                mod._hook = _ntff_profile_via_ctypes("/opt/axon/libaxon_pjrt.so")
        except Exception as e:  # profiling is best-effort
            print("ntff hook shim failed:", e)
            trace = False
    real_maps = [{names[k]: v for k, v in im.items()} for im in in_maps]
    res = bass_utils.run_bass_kernel_spmd(
        nc, real_maps, core_ids=list(range(NC)), trace=trace)
    if trace and res.exec_time_ns is not None:
        print(f"HW exec time: {res.exec_time_ns} ns")
        if res.instructions_and_trace:
            print("trace:", res.instructions_and_trace[1])

    out = np.empty(N_NODES, f32)
    for c in range(NC):
        out[NPN * c:NPN * (c + 1)] = res.results[c][names["zout"]][0, :NPN]
    return out


# revision 8
# speedup vs baseline: 1.4444x; 1.4444x over previous
"""OGRENet GNN message-passing kernel for 8 Trainium2 NeuronCores.

Strategy
--------
Host (numpy, cheap index plumbing only):
  * u2 = u @ Wsel + bsel  (64x256, negligible FLOPs)
  * sort edges by destination node (`row`), split into 8 contiguous chunks at
    node boundaries (2500 nodes per core) -> each core owns a contiguous node
    range and ALL edges that scatter into it => no cross-core reduction.
  * pack per-core feature-major edge inputs ein.T = [x[col]; x[row]; e_attr;
    u2[batch[row]]] (fp16), per-edge 1/count scales, window-relative row ids.

Device (per core, identical program, different data):
  * edge MLP (275->1024x4->512) + node MLP1 (521->512->512) as fp16 matmuls,
    feature-major activations [feat_part, edge_free], fp32 PSUM accumulation.
  * node MLP1 layer2 flips to edge-major [128 edges, 512 feat] so the
    segment-sum becomes a matmul: membership tile M[e, n] = (row[e]==n)
    (built on DVE via is_equal against an iota row) contracts edges away:
    agg.T[f, n] += h.T_tile @ M.  1/count is folded into h via the ACT scale.
  * PSUM accumulates each 256-node window over a static range of edge tiles
    (ranges computed from the actual data on host, shared by all cores;
    membership zeroes any edge outside the window, so overlap is harmless).
  * node MLP2 (777->512->1) consumes agg feature-major directly; z -> DRAM.
"""

import os
import sys

import numpy as np

sys.path.insert(0, "/opt/trn_rl_repo")

N_NODES = 20000
N_GRAPHS = 64
U_DIM = 256
E_HID = 1024
E_OUT = 512
N_HID = 512
NC = 8
NPN = N_NODES // NC          # nodes per core (2500)
NPAD = 2560                  # padded nodes per core
WN = 256                     # nodes per segment window
NWIN = NPAD // WN            # 10
P = 128

_CACHE = {}


def _pack_cols(v, T):
    """[T*128] -> [128, T] with col t = v[t*128:(t+1)*128]."""
    return np.ascontiguousarray(v.reshape(T, P).T)


def _build_module(EPAD, win_tiles, h2_bufs):
    """Build the per-core Bass program. win_tiles[w] = (tlo, thi) inclusive."""
    from concourse import bacc, mybir, tile

    T = EPAD // P           # 128-edge tiles
    NB = EPAD // 1024       # 1024-edge blocks
    f16 = mybir.dt.float16
    f32 = mybir.dt.float32
    RELU = mybir.ActivationFunctionType.Relu
    COPY = mybir.ActivationFunctionType.Copy
    IDENT = mybir.ActivationFunctionType.Identity

    nc = bacc.Bacc(None, target_bir_lowering=False, debug=False)

    with tile.TileContext(nc) as tc:
        with (
            tc.tile_pool(name="dram", bufs=1, space="DRAM") as dram,
            tc.tile_pool(name="wres", bufs=1) as wres,
            tc.tile_pool(name="einp", bufs=5) as einp,
            tc.tile_pool(name="actp", bufs=8) as actp,
            tc.tile_pool(name="act4", bufs=4) as act4,
            tc.tile_pool(name="h2p", bufs=h2_bufs) as h2p,
            tc.tile_pool(name="smal", bufs=3) as smal,
            tc.tile_pool(name="aggp", bufs=8) as aggp,
            tc.tile_pool(name="strm", bufs=4) as strm,
            tc.tile_pool(name="bigps", bufs=3, space="PSUM") as bigps,
            tc.tile_pool(name="segps", bufs=4, space="PSUM") as segps,
        ):
            # ---- DRAM I/O -------------------------------------------------
            d_ein = dram.tile([384, EPAD], f16, kind="ExternalInput", name="ein")
            d_relw = dram.tile([P, NWIN * T], f32, kind="ExternalInput", name="relw")
            d_invc = dram.tile([P, T], f32, kind="ExternalInput", name="invc")
            d_xT2 = dram.tile([P, NPAD], f16, kind="ExternalInput", name="xT2")
            d_u2bT = dram.tile([U_DIM, NPAD], f16, kind="ExternalInput", name="u2bT")
            d_iota = dram.tile([P, WN], f32, kind="ExternalInput", name="iota")
            d_n1b2bc = dram.tile([P, 512], f32, kind="ExternalInput", name="n1b2bc")

            wspec = dict(
                eW1p=[384, E_HID], eW5=[E_HID, E_OUT],
                n1W1a=[P, N_HID], n1W1b=[E_OUT, N_HID], n1W2=[N_HID, N_HID],
                n2W1x=[P, N_HID], n2W1agg=[N_HID, N_HID], n2W1u=[U_DIM, N_HID],
                n2W2=[N_HID, 1],
            )
            d_w = {k: dram.tile(s, f16, kind="ExternalInput", name=k)
                   for k, s in wspec.items()}
            f8 = mybir.dt.float8e4
            for k in ("eW2", "eW3", "eW4"):
                d_w[k] = dram.tile([P, 8192], f8, kind="ExternalInput", name=k)
            bspec = dict(eb1r=[P, 8], eb2r=[P, 8], eb3r=[P, 8], eb4r=[P, 8],
                         eb5r=[P, 4], n1b1r=[P, 4], n2b1r=[P, 4], n2b2r=[1, 1])
            d_b = {k: dram.tile(s, f32, kind="ExternalInput", name=k)
                   for k, s in bspec.items()}
            d_z = dram.tile([1, NPAD], f32, kind="ExternalOutput", name="zout")

            names = dict(ein=d_ein.name, relw=d_relw.name, invc=d_invc.name,
                         xT2=d_xT2.name, u2bT=d_u2bT.name, iota=d_iota.name,
                         n1b2bc=d_n1b2bc.name, zout=d_z.name)
            names.update({k: v.name for k, v in d_w.items()})
            names.update({k: v.name for k, v in d_b.items()})

            # ---- resident loads ------------------------------------------
            def load_w(name, nk, width):
                ts = []
                for k in range(nk):
                    t = wres.tile([P, width], f16, name=f"w_{name}_{k}")
                    nc.sync.dma_start(out=t[:], in_=d_w[name][k * P:(k + 1) * P, :])
                    ts.append(t)
                return ts

            W1 = load_w("eW1p", 3, E_HID)

            def load_wdr(name):
                ts = []
                for q in range(4):
                    t = wres.tile([P, 2, E_HID], f8, name=f"w_{name}_{q}")
                    for j in range(2):
                        nc.sync.dma_start(
                            out=t[:, j, :],
                            in_=d_w[name][:, (q * 2 + j) * E_HID:
                                          (q * 2 + j + 1) * E_HID])
                    ts.append(t)
                return ts

            W2 = load_wdr("eW2")
            W3 = load_wdr("eW3")
            W4 = load_wdr("eW4")
            W5 = load_w("eW5", 8, E_OUT)
            W6a = load_w("n1W1a", 1, N_HID)
            W6b = load_w("n1W1b", 4, N_HID)
            W7 = load_w("n1W2", 4, N_HID)
            W8x = load_w("n2W1x", 1, N_HID)
            W8a = load_w("n2W1agg", 4, N_HID)
            W8u = load_w("n2W1u", 2, N_HID)
            W9 = load_w("n2W2", 4, 1)

            B = {}
            for k, s in bspec.items():
                t = wres.tile(s, f32, name=f"b_{k}")
                nc.sync.dma_start(out=t[:], in_=d_b[k][:])
                B[k] = t
            relw = wres.tile([P, NWIN * T], f32, name="relw_sb")
            nc.sync.dma_start(out=relw[:], in_=d_relw[:])
            invc = wres.tile([P, T], f32, name="invc_sb")
            nc.sync.dma_start(out=invc[:], in_=d_invc[:])
            iota = wres.tile([P, WN], f32, name="iota_sb")
            nc.sync.dma_start(out=iota[:], in_=d_iota[:])
            n1b2bc = wres.tile([P, 512], f32, name="n1b2bc_sb")
            nc.sync.dma_start(out=n1b2bc[:], in_=d_n1b2bc[:])
            xT2 = wres.tile([P, NPAD], f16, name="xT2_sb")
            nc.sync.dma_start(out=xT2[:], in_=d_xT2[:])

            # ---- helpers --------------------------------------------------
            def fm_layer(ins, Wt, bias, nM, width, act, out_pool, tag):
                """feature-major layer: out[m] [128, width_edges]"""
                outs = []
                for m in range(nM):
                    ps = bigps.tile([P, 512], mybir.dt.float32, name="ps_big")
                    pss = ps[:, :width]
                    for ki, (it, wt) in enumerate(zip(ins, Wt)):
                        nc.tensor.matmul(
                            out=pss, lhsT=wt[:, m * P:(m + 1) * P], rhs=it,
                            start=(ki == 0), stop=(ki == len(ins) - 1))
                    ot = out_pool.tile([P, width], f16, name=tag)
                    nc.scalar.activation(ot[:], pss, act, bias=bias[:, m:m + 1])
                    outs.append(ot)
                return outs

            DR = mybir.MatmulPerfMode.DoubleRow
            INV64 = 1.0 / 64.0

            def dr_layer(pin, Wp, bias, scale, pair_out, tag):
                """fp8 DoubleRow layer: pin = 4 pair tiles [128,2,512]."""
                outs = []
                pt = None
                for m in range(8):
                    ps = bigps.tile([P, 512], mybir.dt.float32, name="ps_big")
                    for q in range(4):
                        nc.tensor.matmul(
                            out=ps[:], lhsT=Wp[q][:, :, m * P:(m + 1) * P],
                            rhs=pin[q][:, :, :], start=(q == 0), stop=(q == 3),
                            perf_mode=DR)
                    if pair_out:
                        if m % 2 == 0:
                            pt = actp.tile([P, 2, 512], f8, name=tag)
                            outs.append(pt)
                        nc.scalar.activation(pt[:, m % 2, :], ps[:], RELU,
                                             bias=bias[:, m:m + 1], scale=scale)
                    else:
                        ot = actp.tile([P, 512], f16, name=tag)
                        nc.scalar.activation(ot[:], ps[:], RELU,
                                             bias=bias[:, m:m + 1], scale=scale)
                        outs.append(ot)
                return outs

            # h2 tiles by global tile index
            h2_tiles = {}

            def emit_block(b):
                e0 = b * 1024
                ein = []
                for k in range(3):
                    t = einp.tile([P, 1024], f16, name="ein_t")
                    nc.sync.dma_start(out=t[:], in_=d_ein[k * P:(k + 1) * P, e0:e0 + 1024])
                    ein.append(t)
                for h in range(2):
                    hs = slice(h * 512, (h + 1) * 512)
                    einh = [e[:, hs] for e in ein]
                    a1p = []
                    pt = None
                    for m in range(8):
                        ps = bigps.tile([P, 512], mybir.dt.float32, name="ps_big")
                        for ki in range(3):
                            nc.tensor.matmul(
                                out=ps[:], lhsT=W1[ki][:, m * P:(m + 1) * P],
                                rhs=einh[ki], start=(ki == 0), stop=(ki == 2))
                        if m % 2 == 0:
                            pt = actp.tile([P, 2, 512], f8, name="pairA")
                            a1p.append(pt)
                        nc.scalar.activation(pt[:, m % 2, :], ps[:], RELU,
                                             bias=B["eb1r"][:, m:m + 1])
                    a2p = dr_layer(a1p, W2, B["eb2r"], INV64, True, "pairB")
                    a3p = dr_layer(a2p, W3, B["eb3r"], INV64, True, "pairA")
                    a4 = dr_layer(a3p, W4, B["eb4r"], INV64, False, "actB")
                    e5 = fm_layer([a[:] for a in a4], W5, B["eb5r"], 4, 512,
                                  IDENT, act4, "actC")
                    h1 = fm_layer([einh[0]] + [a[:] for a in e5],
                                  [W6a[0]] + W6b, B["n1b1r"], 4, 512, RELU,
                                  act4, "actD")
                    # n1 layer2 -> edge-major h2 per 128-edge subtile, then
                    # bias-add (DVE) + relu*invc (ACT) -> fp16
                    for s in range(4):
                        t_glob = b * 8 + h * 4 + s
                        ps = bigps.tile([P, 512], mybir.dt.float32, name="ps_big")
                        pss = ps[:, :512]
                        for ki in range(4):
                            nc.tensor.matmul(
                                out=pss, lhsT=h1[ki][:, s * P:(s + 1) * P],
                                rhs=W7[ki][:], start=(ki == 0), stop=(ki == 3))
                        tmp = smal.tile([P, 512], f16, name="tmp16")
                        nc.vector.tensor_tensor(out=tmp[:], in0=pss, in1=n1b2bc[:],
                                                op=mybir.AluOpType.add)
                        h2 = h2p.tile([P, 512], f16, name="h2t")
                        nc.scalar.activation(h2[:], tmp[:], RELU,
                                             scale=invc[:, t_glob:t_glob + 1])
                        h2_tiles[t_glob] = h2

            def emit_window(w):
                tlo, thi = win_tiles[w]
                seg = [segps.tile([P, WN], mybir.dt.float32, name="segps_t")
                       for _ in range(4)]
                tl = list(range(tlo, thi + 1))
                for si, t in enumerate(tl):
                    memb = smal.tile([P, WN], f16, name="memb")
                    nc.vector.tensor_scalar(
                        out=memb[:], in0=iota[:],
                        scalar1=relw[:, w * T + t:w * T + t + 1], scalar2=None,
                        op0=mybir.AluOpType.is_equal)
                    h2 = h2_tiles[t]
                    for fc in range(4):
                        nc.tensor.matmul(
                            out=seg[fc][:], lhsT=h2[:, fc * P:(fc + 1) * P],
                            rhs=memb[:], start=(si == 0), stop=(si == len(tl) - 1))
                agg = []
                for fc in range(4):
                    at = aggp.tile([P, WN], f16, name="aggt")
                    nc.scalar.activation(at[:], seg[fc][:], COPY)
                    agg.append(at)
                # ---- node MLP2 on this 256-node window ----
                n0 = w * WN
                u2c = []
                for k in range(2):
                    t = strm.tile([P, WN], f16, name="u2c")
                    nc.sync.dma_start(out=t[:], in_=d_u2bT[k * P:(k + 1) * P,
                                                          n0:n0 + WN])
                    u2c.append(t)
                xin = [xT2[:, n0:n0 + WN]] + [a[:] for a in agg] + \
                      [u2c[0][:], u2c[1][:]]
                Win = [W8x[0]] + W8a + W8u
                z1 = []
                for m in range(4):
                    ps = bigps.tile([P, 512], mybir.dt.float32, name="ps_big")
                    pss = ps[:, :WN]
                    for ki in range(7):
                        nc.tensor.matmul(out=pss, lhsT=Win[ki][:, m * P:(m + 1) * P],
                                         rhs=xin[ki], start=(ki == 0), stop=(ki == 6))
                    zt = smal.tile([P, WN], f16, name="z1t")
                    nc.scalar.activation(zt[:], pss, RELU,
                                         bias=B["n2b1r"][:, m:m + 1])
                    z1.append(zt)
                ps = bigps.tile([P, 512], mybir.dt.float32, name="ps_big")
                pss = ps[:1, :WN]
                for ki in range(4):
                    nc.tensor.matmul(out=pss, lhsT=W9[ki][:], rhs=z1[ki][:],
                                     start=(ki == 0), stop=(ki == 3))
                zo = smal.tile([1, WN], f32, name="zot")
                nc.scalar.activation(zo[:], pss, IDENT, bias=B["n2b2r"][:])
                nc.sync.dma_start(out=d_z[:, n0:n0 + WN], in_=zo[:])

            # window w ready once tile win_tiles[w][1] is produced
            ready = {}
            for w in range(NWIN):
                b_ready = min(NB - 1, win_tiles[w][1] // 8)
                ready.setdefault(b_ready, []).append(w)
            for b in range(NB):
                emit_block(b)
                for w in ready.get(b, []):
                    emit_window(w)

    nc.compile()
    return nc, names


def kernel(x, edge_attr, u, edge_index, batch, Wsel, bsel,
           eW1, eb1, eW2, eb2, eW3, eb3, eW4, eb4, eW5, eb5,
           n1W1, n1b1, n1W2, n1b2, n2W1, n2b1, n2W2, n2b2):
    f32 = np.float32
    f16 = np.float16
    x = np.asarray(x, f32)
    edge_attr = np.asarray(edge_attr, f32)
    u = np.asarray(u, f32)
    edge_index = np.asarray(edge_index)
    batch = np.asarray(batch)
    ws = {k: np.asarray(v, f32) for k, v in dict(
        Wsel=Wsel, bsel=bsel, eW1=eW1, eb1=eb1, eW2=eW2, eb2=eb2, eW3=eW3,
        eb3=eb3, eW4=eW4, eb4=eb4, eW5=eW5, eb5=eb5, n1W1=n1W1, n1b1=n1b1,
        n1W2=n1W2, n1b2=n1b2, n2W1=n2W1, n2b1=n2b1, n2W2=n2W2, n2b2=n2b2).items()}

    # ---------------- host math (index plumbing + tiny matmul) -------------
    u2 = (u @ ws["Wsel"] + ws["bsel"]).astype(f32)          # [64, 256]
    row = np.asarray(edge_index[0], np.int64)
    col = np.asarray(edge_index[1], np.int64)
    order = np.argsort(row, kind="stable")
    row_s, col_s = row[order], col[order]
    g_s = batch[row_s]
    ea_s = edge_attr[order, 0]
    cnt = np.bincount(row, minlength=N_NODES).astype(f32)
    invc_node = (1.0 / np.maximum(cnt, 1.0)).astype(f32)

    bounds = np.searchsorted(row_s, np.arange(0, N_NODES + 1, NPN))
    e_cnt = np.diff(bounds)
    EPAD = int(-(-int(e_cnt.max()) // 1024) * 1024)
    T = EPAD // P

    # per-window tile ranges (shared across cores) for the static program
    tlo = np.full(NWIN, T - 1, np.int64)
    thi = np.zeros(NWIN, np.int64)
    core_dat = []
    for c in range(NC):
        lo, hi = bounds[c], bounds[c + 1]
        n = hi - lo
        rel = np.full(EPAD, 1e6, f32)
        rel[:n] = (row_s[lo:hi] - NPN * c).astype(f32)
        w_of_edge = np.floor_divide(rel[:n].astype(np.int64), WN)
        for w in range(NWIN):
            idx = np.nonzero(w_of_edge == w)[0]
            if idx.size:
                tlo[w] = min(tlo[w], idx[0] // P)
                thi[w] = max(thi[w], idx[-1] // P)
        core_dat.append((lo, hi, n, rel))
    win_tiles = [(int(tlo[w]), int(max(tlo[w], thi[w]))) for w in range(NWIN)]
    h2_bufs = max(hw - lw + 1 for lw, hw in win_tiles) + 14

    # ---------------- per-core input packing --------------------------------
    zpad109 = np.zeros((109, ws["eW1"].shape[1]), f32)
    eW1p = np.concatenate([ws["eW1"][9:18], ws["eW1"][0:9], ws["eW1"][18:19],
                           ws["eW1"][19:275], zpad109], axis=0)
    n1W1a = np.zeros((P, N_HID), f32)
    n1W1a[0:9] = ws["n1W1"][0:9]
    n2W1x = np.zeros((P, N_HID), f32)
    n2W1x[0:9] = ws["n2W1"][0:9]

    def br(b, nm):   # bias [nm*128] -> [128, nm]
        return np.ascontiguousarray(b.reshape(nm, P).T).astype(f32)

    import ml_dtypes
    fp8 = ml_dtypes.float8_e4m3

    def packdr(W):   # [1024,1024] -> [128, 8192] fp8, x64, (q,j,m) free order
        Wp = (W * 64.0).reshape(4, 2, P, E_HID)
        return np.ascontiguousarray(
            np.transpose(Wp, (2, 0, 1, 3)).reshape(P, 8192)).astype(fp8)

    shared = dict(
        eW1p=eW1p.astype(f16), eW2=packdr(ws["eW2"]),
        eW3=packdr(ws["eW3"]), eW4=packdr(ws["eW4"]),
        eW5=ws["eW5"].astype(f16), n1W1a=n1W1a.astype(f16),
        n1W1b=ws["n1W1"][9:521].astype(f16), n1W2=ws["n1W2"].astype(f16),
        n2W1x=n2W1x.astype(f16), n2W1agg=ws["n2W1"][9:521].astype(f16),
        n2W1u=ws["n2W1"][521:777].astype(f16), n2W2=ws["n2W2"].astype(f16),
        eb1r=br(ws["eb1"], 8), eb2r=br(ws["eb2"], 8), eb3r=br(ws["eb3"], 8),
        eb4r=br(ws["eb4"], 8), eb5r=br(ws["eb5"], 4), n1b1r=br(ws["n1b1"], 4),
        n2b1r=br(ws["n2b1"], 4), n2b2r=ws["n2b2"].reshape(1, 1).astype(f32),
        iota=np.tile(np.arange(WN, dtype=f32), (P, 1)),
        n1b2bc=np.tile(ws["n1b2"].astype(f32), (P, 1)),
    )

    in_maps = []
    for c in range(NC):
        lo, hi, n, rel = core_dat[c]
        ein = np.zeros((384, EPAD), f16)
        ein[0:9, :n] = x[col_s[lo:hi]].T
        ein[9:18, :n] = x[row_s[lo:hi]].T
        ein[18, :n] = ea_s[lo:hi]
        ein[19:275, :n] = u2[g_s[lo:hi]].T
        relw = np.empty((P, NWIN * T), f32)
        for w in range(NWIN):
            relw[:, w * T:(w + 1) * T] = _pack_cols(rel - float(WN) * w, T)
        invc_e = np.ones(EPAD, f32)
        invc_e[:n] = invc_node[row_s[lo:hi]]
        xT2 = np.zeros((P, NPAD), f16)
        xT2[0:9, :NPN] = x[NPN * c:NPN * (c + 1)].T
        u2bT = np.zeros((U_DIM, NPAD), f16)
        u2bT[:, :NPN] = u2[batch[NPN * c:NPN * (c + 1)]].T
        im = dict(shared)
        im.update(ein=ein, relw=relw, invc=_pack_cols(invc_e, T),
                  xT2=xT2, u2bT=u2bT)
        in_maps.append(im)

    # ---------------- build + run ------------------------------------------
    key = (EPAD, tuple(win_tiles))
    if key not in _CACHE:
        _CACHE[key] = _build_module(EPAD, win_tiles, h2_bufs)
    nc, names = _CACHE[key]

    from concourse import bass_utils
    trace = bool(int(os.environ.get("KERNEL_TRACE", "0")))
    if trace:
        try:
            import types
            import antenv
            if not hasattr(antenv, "axon_hooks"):
                mod = types.ModuleType("antenv.axon_hooks")
                mod._hook = None
                mod.set_axon_ntff_profile_hook = lambda h: setattr(mod, "_hook", h)
                mod.get_axon_ntff_profile_hook = lambda: mod._hook
                sys.modules["antenv.axon_hooks"] = mod
                antenv.axon_hooks = mod
                from trn_agent_boot.trn_boot import _ntff_profile_via_ctypes
                mod._hook = _ntff_profile_via_ctypes("/opt/axon/libaxon_pjrt.so")
        except Exception as e:  # profiling is best-effort
            print("ntff hook shim failed:", e)
            trace = False
    real_maps = [{names[k]: v for k, v in im.items()} for im in in_maps]
    res = bass_utils.run_bass_kernel_spmd(
        nc, real_maps, core_ids=list(range(NC)), trace=trace)
    if trace and res.exec_time_ns is not None:
        print(f"HW exec time: {res.exec_time_ns} ns")
        if res.instructions_and_trace:
            print("trace:", res.instructions_and_trace[1])

    out = np.empty(N_NODES, f32)
    for c in range(NC):
        out[NPN * c:NPN * (c + 1)] = res.results[c][names["zout"]][0, :NPN]
    return out


# revision 10
# speedup vs baseline: 1.6574x; 1.1475x over previous
"""OGRENet GNN message-passing kernel for 8 Trainium2 NeuronCores.

Strategy
--------
Host (numpy, cheap index plumbing only):
  * u2 = u @ Wsel + bsel  (64x256, negligible FLOPs)
  * sort edges by destination node (`row`), split into 8 contiguous chunks at
    node boundaries (2500 nodes per core) -> each core owns a contiguous node
    range and ALL edges that scatter into it => no cross-core reduction.
  * pack per-core feature-major edge inputs ein.T = [x[col]; x[row]; e_attr;
    u2[batch[row]]] (fp16), per-edge 1/count scales, window-relative row ids.

Device (per core, identical program, different data):
  * edge MLP (275->1024x4->512) + node MLP1 (521->512->512) as fp16 matmuls,
    feature-major activations [feat_part, edge_free], fp32 PSUM accumulation.
  * node MLP1 layer2 flips to edge-major [128 edges, 512 feat] so the
    segment-sum becomes a matmul: membership tile M[e, n] = (row[e]==n)
    (built on DVE via is_equal against an iota row) contracts edges away:
    agg.T[f, n] += h.T_tile @ M.  1/count is folded into h via the ACT scale.
  * PSUM accumulates each 256-node window over a static range of edge tiles
    (ranges computed from the actual data on host, shared by all cores;
    membership zeroes any edge outside the window, so overlap is harmless).
  * node MLP2 (777->512->1) consumes agg feature-major directly; z -> DRAM.
"""

import os
import sys

import numpy as np

sys.path.insert(0, "/opt/trn_rl_repo")

N_NODES = 20000
N_GRAPHS = 64
U_DIM = 256
E_HID = 1024
E_OUT = 512
N_HID = 512
NC = 8
NPN = N_NODES // NC          # nodes per core (2500)
NPAD = 2560                  # padded nodes per core
WN = 256                     # nodes per segment window
NWIN = NPAD // WN            # 10
P = 128

_CACHE = {}


def _pack_cols(v, T):
    """[T*128] -> [128, T] with col t = v[t*128:(t+1)*128]."""
    return np.ascontiguousarray(v.reshape(T, P).T)


def _build_module(EPAD, win_tiles, h2_bufs):
    """Build the per-core Bass program. win_tiles[w] = (tlo, thi) inclusive."""
    from concourse import bacc, mybir, tile

    T = EPAD // P           # 128-edge tiles
    NB = EPAD // 1024       # 1024-edge blocks
    f16 = mybir.dt.float16
    f32 = mybir.dt.float32
    RELU = mybir.ActivationFunctionType.Relu
    COPY = mybir.ActivationFunctionType.Copy
    IDENT = mybir.ActivationFunctionType.Identity

    nc = bacc.Bacc(None, target_bir_lowering=False, debug=False)

    with tile.TileContext(nc) as tc:
        with (
            tc.tile_pool(name="dram", bufs=1, space="DRAM") as dram,
            tc.tile_pool(name="wres", bufs=1) as wres,
            tc.tile_pool(name="einp", bufs=5) as einp,
            tc.tile_pool(name="actp", bufs=8) as actp,
            tc.tile_pool(name="act4", bufs=4) as act4,
            tc.tile_pool(name="h2p", bufs=h2_bufs) as h2p,
            tc.tile_pool(name="smal", bufs=3) as smal,
            tc.tile_pool(name="aggp", bufs=8) as aggp,
            tc.tile_pool(name="strm", bufs=4) as strm,
            tc.tile_pool(name="bigps", bufs=3, space="PSUM") as bigps,
            tc.tile_pool(name="segps", bufs=4, space="PSUM") as segps,
        ):
            # ---- DRAM I/O -------------------------------------------------
            d_ein = dram.tile([384, EPAD], f16, kind="ExternalInput", name="ein")
            d_relw = dram.tile([P, NWIN * T], f32, kind="ExternalInput", name="relw")
            d_invc = dram.tile([P, T], f32, kind="ExternalInput", name="invc")
            d_xT2 = dram.tile([P, NPAD], f16, kind="ExternalInput", name="xT2")
            d_u2bT = dram.tile([U_DIM, NPAD], f16, kind="ExternalInput", name="u2bT")
            d_iota = dram.tile([P, WN], f32, kind="ExternalInput", name="iota")
            d_n1b2bc = dram.tile([P, 512], f32, kind="ExternalInput", name="n1b2bc")

            wspec = dict(
                eW1p=[384, E_HID],
                n1W1a=[P, N_HID],
                n2W1x=[P, N_HID], n2W1agg=[N_HID, N_HID], n2W1u=[U_DIM, N_HID],
                n2W2=[N_HID, 1],
            )
            d_w = {k: dram.tile(s, f16, kind="ExternalInput", name=k)
                   for k, s in wspec.items()}
            f8 = mybir.dt.float8e4
            for k in ("eW2", "eW3", "eW4"):
                d_w[k] = dram.tile([P, 8192], f8, kind="ExternalInput", name=k)
            d_w["eW5"] = dram.tile([P, 4096], f8, kind="ExternalInput", name="eW5")
            d_w["n1W1b"] = dram.tile([P, 2048], f8, kind="ExternalInput", name="n1W1b")
            d_w["n1W2"] = dram.tile([P, 2048], f8, kind="ExternalInput", name="n1W2")
            bspec = dict(eb1r=[P, 8], eb2r=[P, 8], eb3r=[P, 8], eb4r=[P, 8],
                         eb5r=[P, 4], n1b1r=[P, 4], n2b1r=[P, 4], n2b2r=[1, 1])
            d_b = {k: dram.tile(s, f32, kind="ExternalInput", name=k)
                   for k, s in bspec.items()}
            d_z = dram.tile([1, NPAD], f32, kind="ExternalOutput", name="zout")

            names = dict(ein=d_ein.name, relw=d_relw.name, invc=d_invc.name,
                         xT2=d_xT2.name, u2bT=d_u2bT.name, iota=d_iota.name,
                         n1b2bc=d_n1b2bc.name, zout=d_z.name)
            names.update({k: v.name for k, v in d_w.items()})
            names.update({k: v.name for k, v in d_b.items()})

            # ---- resident loads ------------------------------------------
            def load_w(name, nk, width):
                ts = []
                for k in range(nk):
                    t = wres.tile([P, width], f16, name=f"w_{name}_{k}")
                    nc.sync.dma_start(out=t[:], in_=d_w[name][k * P:(k + 1) * P, :])
                    ts.append(t)
                return ts

            W1 = load_w("eW1p", 3, E_HID)

            def load_wdr(name, npair, width):
                ts = []
                for q in range(npair):
                    t = wres.tile([P, 2, width], f8, name=f"w_{name}_{q}")
                    for j in range(2):
                        nc.sync.dma_start(
                            out=t[:, j, :],
                            in_=d_w[name][:, (q * 2 + j) * width:
                                          (q * 2 + j + 1) * width])
                    ts.append(t)
                return ts

            W2 = load_wdr("eW2", 4, E_HID)
            W3 = load_wdr("eW3", 4, E_HID)
            W4 = load_wdr("eW4", 4, E_HID)
            W5 = load_wdr("eW5", 4, E_OUT)
            W6a = load_w("n1W1a", 1, N_HID)
            W6b = load_wdr("n1W1b", 2, N_HID)
            W7 = load_wdr("n1W2", 2, N_HID)
            W8x = load_w("n2W1x", 1, N_HID)
            W8a = load_w("n2W1agg", 4, N_HID)
            W8u = load_w("n2W1u", 2, N_HID)
            W9 = load_w("n2W2", 4, 1)

            B = {}
            for k, s in bspec.items():
                t = wres.tile(s, f32, name=f"b_{k}")
                nc.sync.dma_start(out=t[:], in_=d_b[k][:])
                B[k] = t
            relw = wres.tile([P, NWIN * T], f32, name="relw_sb")
            nc.sync.dma_start(out=relw[:], in_=d_relw[:])
            invc = wres.tile([P, T], f32, name="invc_sb")
            nc.sync.dma_start(out=invc[:], in_=d_invc[:])
            iota = wres.tile([P, WN], f32, name="iota_sb")
            nc.sync.dma_start(out=iota[:], in_=d_iota[:])
            n1b2bc = wres.tile([P, 512], f32, name="n1b2bc_sb")
            nc.sync.dma_start(out=n1b2bc[:], in_=d_n1b2bc[:])
            xT2 = wres.tile([P, NPAD], f16, name="xT2_sb")
            nc.sync.dma_start(out=xT2[:], in_=d_xT2[:])

            # ---- helpers --------------------------------------------------
            def fm_layer(ins, Wt, bias, nM, width, act, out_pool, tag):
                """feature-major layer: out[m] [128, width_edges]"""
                outs = []
                for m in range(nM):
                    ps = bigps.tile([P, 512], mybir.dt.float32, name="ps_big")
                    pss = ps[:, :width]
                    for ki, (it, wt) in enumerate(zip(ins, Wt)):
                        nc.tensor.matmul(
                            out=pss, lhsT=wt[:, m * P:(m + 1) * P], rhs=it,
                            start=(ki == 0), stop=(ki == len(ins) - 1))
                    ot = out_pool.tile([P, width], f16, name=tag)
                    nc.scalar.activation(ot[:], pss, act, bias=bias[:, m:m + 1])
                    outs.append(ot)
                return outs

            DR = mybir.MatmulPerfMode.DoubleRow
            INV64 = 1.0 / 64.0

            def dr_layer(pin, Wp, bias, scale, nM, act, pair_out, tag):
                """fp8 DoubleRow layer: pin = pair tiles [128,2,512]."""
                outs = []
                pt = None
                for m in range(nM):
                    ps = bigps.tile([P, 512], mybir.dt.float32, name="ps_big")
                    for q in range(len(Wp)):
                        nc.tensor.matmul(
                            out=ps[:], lhsT=Wp[q][:, :, m * P:(m + 1) * P],
                            rhs=pin[q][:, :, :], start=(q == 0),
                            stop=(q == len(Wp) - 1), perf_mode=DR)
                    if pair_out:
                        if m % 2 == 0:
                            pt = actp.tile([P, 2, 512], f8, name=tag)
                            outs.append(pt)
                        nc.scalar.activation(pt[:, m % 2, :], ps[:], act,
                                             bias=bias[:, m:m + 1], scale=scale)
                    else:
                        ot = actp.tile([P, 512], f16, name=tag)
                        nc.scalar.activation(ot[:], ps[:], act,
                                             bias=bias[:, m:m + 1], scale=scale)
                        outs.append(ot)
                return outs

            # h2 tiles by global tile index
            h2_tiles = {}

            def emit_block(b):
                e0 = b * 1024
                ein = []
                for k in range(3):
                    t = einp.tile([P, 1024], f16, name="ein_t")
                    nc.sync.dma_start(out=t[:], in_=d_ein[k * P:(k + 1) * P, e0:e0 + 1024])
                    ein.append(t)
                for h in range(2):
                    hs = slice(h * 512, (h + 1) * 512)
                    einh = [e[:, hs] for e in ein]
                    a1p = []
                    pt = None
                    for m in range(8):
                        ps = bigps.tile([P, 512], mybir.dt.float32, name="ps_big")
                        for ki in range(3):
                            nc.tensor.matmul(
                                out=ps[:], lhsT=W1[ki][:, m * P:(m + 1) * P],
                                rhs=einh[ki], start=(ki == 0), stop=(ki == 2))
                        if m % 2 == 0:
                            pt = actp.tile([P, 2, 512], f8, name="pairA")
                            a1p.append(pt)
                        nc.scalar.activation(pt[:, m % 2, :], ps[:], RELU,
                                             bias=B["eb1r"][:, m:m + 1])
                    a2p = dr_layer(a1p, W2, B["eb2r"], INV64, 8, RELU, True, "pairB")
                    a3p = dr_layer(a2p, W3, B["eb3r"], INV64, 8, RELU, True, "pairA")
                    a4p = dr_layer(a3p, W4, B["eb4r"], INV64, 8, RELU, True, "pairB")
                    e5p = dr_layer(a4p, W5, B["eb5r"], INV64, 4, IDENT, True, "pairC")
                    # n1 L1: fp16 x-part (x64 weights) + fp8 DR e-part
                    h1p = []
                    pt = None
                    for m in range(4):
                        ps = bigps.tile([P, 512], mybir.dt.float32, name="ps_big")
                        nc.tensor.matmul(
                            out=ps[:], lhsT=W6a[0][:, m * P:(m + 1) * P],
                            rhs=einh[0], start=True, stop=False)
                        for q in range(2):
                            nc.tensor.matmul(
                                out=ps[:], lhsT=W6b[q][:, :, m * P:(m + 1) * P],
                                rhs=e5p[q][:, :, :], start=False, stop=(q == 1),
                                perf_mode=DR)
                        if m % 2 == 0:
                            pt = actp.tile([P, 2, 512], f8, name="pairD")
                            h1p.append(pt)
                        nc.scalar.activation(pt[:, m % 2, :], ps[:], RELU,
                                             bias=B["n1b1r"][:, m:m + 1],
                                             scale=INV64)
                    # n1 layer2 -> edge-major h2 per 128-edge subtile, then
                    # bias-add (DVE, x64 domain) + relu*(invc/64) (ACT) -> fp16
                    for s in range(4):
                        t_glob = b * 8 + h * 4 + s
                        ps = bigps.tile([P, 512], mybir.dt.float32, name="ps_big")
                        pss = ps[:, :512]
                        for q in range(2):
                            nc.tensor.matmul(
                                out=pss, lhsT=h1p[q][:, :, s * P:(s + 1) * P],
                                rhs=W7[q][:, :, :], start=(q == 0), stop=(q == 1),
                                perf_mode=DR)
                        tmp = smal.tile([P, 512], f16, name="tmp16")
                        nc.vector.tensor_tensor(out=tmp[:], in0=pss, in1=n1b2bc[:],
                                                op=mybir.AluOpType.add)
                        h2 = h2p.tile([P, 512], f16, name="h2t")
                        nc.scalar.activation(h2[:], tmp[:], RELU,
                                             scale=invc[:, t_glob:t_glob + 1])
                        h2_tiles[t_glob] = h2

            def emit_window(w):
                tlo, thi = win_tiles[w]
                seg = [segps.tile([P, WN], mybir.dt.float32, name="segps_t")
                       for _ in range(4)]
                tl = list(range(tlo, thi + 1))
                for si, t in enumerate(tl):
                    memb = smal.tile([P, WN], f16, name="memb")
                    nc.vector.tensor_scalar(
                        out=memb[:], in0=iota[:],
                        scalar1=relw[:, w * T + t:w * T + t + 1], scalar2=None,
                        op0=mybir.AluOpType.is_equal)
                    h2 = h2_tiles[t]
                    for fc in range(4):
                        nc.tensor.matmul(
                            out=seg[fc][:], lhsT=h2[:, fc * P:(fc + 1) * P],
                            rhs=memb[:], start=(si == 0), stop=(si == len(tl) - 1))
                agg = []
                for fc in range(4):
                    at = aggp.tile([P, WN], f16, name="aggt")
                    nc.scalar.activation(at[:], seg[fc][:], COPY)
                    agg.append(at)
                # ---- node MLP2 on this 256-node window ----
                n0 = w * WN
                u2c = []
                for k in range(2):
                    t = strm.tile([P, WN], f16, name="u2c")
                    nc.sync.dma_start(out=t[:], in_=d_u2bT[k * P:(k + 1) * P,
                                                          n0:n0 + WN])
                    u2c.append(t)
                xin = [xT2[:, n0:n0 + WN]] + [a[:] for a in agg] + \
                      [u2c[0][:], u2c[1][:]]
                Win = [W8x[0]] + W8a + W8u
                z1 = []
                for m in range(4):
                    ps = bigps.tile([P, 512], mybir.dt.float32, name="ps_big")
                    pss = ps[:, :WN]
                    for ki in range(7):
                        nc.tensor.matmul(out=pss, lhsT=Win[ki][:, m * P:(m + 1) * P],
                                         rhs=xin[ki], start=(ki == 0), stop=(ki == 6))
                    zt = smal.tile([P, WN], f16, name="z1t")
                    nc.scalar.activation(zt[:], pss, RELU,
                                         bias=B["n2b1r"][:, m:m + 1])
                    z1.append(zt)
                ps = bigps.tile([P, 512], mybir.dt.float32, name="ps_big")
                pss = ps[:1, :WN]
                for ki in range(4):
                    nc.tensor.matmul(out=pss, lhsT=W9[ki][:], rhs=z1[ki][:],
                                     start=(ki == 0), stop=(ki == 3))
                zo = smal.tile([1, WN], f32, name="zot")
                nc.scalar.activation(zo[:], pss, IDENT, bias=B["n2b2r"][:])
                nc.sync.dma_start(out=d_z[:, n0:n0 + WN], in_=zo[:])

            # window w ready once tile win_tiles[w][1] is produced
            ready = {}
            for w in range(NWIN):
                b_ready = min(NB - 1, win_tiles[w][1] // 8)
                ready.setdefault(b_ready, []).append(w)
            for b in range(NB):
                emit_block(b)
                for w in ready.get(b, []):
                    emit_window(w)

    nc.compile()
    return nc, names


def kernel(x, edge_attr, u, edge_index, batch, Wsel, bsel,
           eW1, eb1, eW2, eb2, eW3, eb3, eW4, eb4, eW5, eb5,
           n1W1, n1b1, n1W2, n1b2, n2W1, n2b1, n2W2, n2b2):
    f32 = np.float32
    f16 = np.float16
    x = np.asarray(x, f32)
    edge_attr = np.asarray(edge_attr, f32)
    u = np.asarray(u, f32)
    edge_index = np.asarray(edge_index)
    batch = np.asarray(batch)
    ws = {k: np.asarray(v, f32) for k, v in dict(
        Wsel=Wsel, bsel=bsel, eW1=eW1, eb1=eb1, eW2=eW2, eb2=eb2, eW3=eW3,
        eb3=eb3, eW4=eW4, eb4=eb4, eW5=eW5, eb5=eb5, n1W1=n1W1, n1b1=n1b1,
        n1W2=n1W2, n1b2=n1b2, n2W1=n2W1, n2b1=n2b1, n2W2=n2W2, n2b2=n2b2).items()}

    # ---------------- host math (index plumbing + tiny matmul) -------------
    u2 = (u @ ws["Wsel"] + ws["bsel"]).astype(f32)          # [64, 256]
    row = np.asarray(edge_index[0], np.int64)
    col = np.asarray(edge_index[1], np.int64)
    order = np.argsort(row, kind="stable")
    row_s, col_s = row[order], col[order]
    g_s = batch[row_s]
    ea_s = edge_attr[order, 0]
    cnt = np.bincount(row, minlength=N_NODES).astype(f32)
    invc_node = (1.0 / np.maximum(cnt, 1.0)).astype(f32)

    bounds = np.searchsorted(row_s, np.arange(0, N_NODES + 1, NPN))
    e_cnt = np.diff(bounds)
    EPAD = int(-(-int(e_cnt.max()) // 1024) * 1024)
    T = EPAD // P

    # per-window tile ranges (shared across cores) for the static program
    tlo = np.full(NWIN, T - 1, np.int64)
    thi = np.zeros(NWIN, np.int64)
    core_dat = []
    for c in range(NC):
        lo, hi = bounds[c], bounds[c + 1]
        n = hi - lo
        rel = np.full(EPAD, 1e6, f32)
        rel[:n] = (row_s[lo:hi] - NPN * c).astype(f32)
        w_of_edge = np.floor_divide(rel[:n].astype(np.int64), WN)
        for w in range(NWIN):
            idx = np.nonzero(w_of_edge == w)[0]
            if idx.size:
                tlo[w] = min(tlo[w], idx[0] // P)
                thi[w] = max(thi[w], idx[-1] // P)
        core_dat.append((lo, hi, n, rel))
    win_tiles = [(int(tlo[w]), int(max(tlo[w], thi[w]))) for w in range(NWIN)]
    h2_bufs = max(hw - lw + 1 for lw, hw in win_tiles) + 14

    # ---------------- per-core input packing --------------------------------
    zpad109 = np.zeros((109, ws["eW1"].shape[1]), f32)
    eW1p = np.concatenate([ws["eW1"][9:18], ws["eW1"][0:9], ws["eW1"][18:19],
                           ws["eW1"][19:275], zpad109], axis=0)
    n1W1a = np.zeros((P, N_HID), f32)
    n1W1a[0:9] = ws["n1W1"][0:9] * 64.0
    n2W1x = np.zeros((P, N_HID), f32)
    n2W1x[0:9] = ws["n2W1"][0:9]

    def br(b, nm):   # bias [nm*128] -> [128, nm]
        return np.ascontiguousarray(b.reshape(nm, P).T).astype(f32)

    import ml_dtypes
    fp8 = ml_dtypes.float8_e4m3

    def packdr(W):   # [K, M] -> [128, (K//128)*M] fp8, x64, (q,j,m) free order
        K, M = W.shape
        Wp = (W * 64.0).reshape(K // 256, 2, P, M)
        return np.ascontiguousarray(
            np.transpose(Wp, (2, 0, 1, 3)).reshape(P, (K // P) * M)).astype(fp8)

    shared = dict(
        eW1p=eW1p.astype(f16), eW2=packdr(ws["eW2"]),
        eW3=packdr(ws["eW3"]), eW4=packdr(ws["eW4"]),
        eW5=packdr(ws["eW5"]), n1W1a=n1W1a.astype(f16),
        n1W1b=packdr(ws["n1W1"][9:521]), n1W2=packdr(ws["n1W2"]),
        n2W1x=n2W1x.astype(f16), n2W1agg=ws["n2W1"][9:521].astype(f16),
        n2W1u=ws["n2W1"][521:777].astype(f16), n2W2=ws["n2W2"].astype(f16),
        eb1r=br(ws["eb1"], 8), eb2r=br(ws["eb2"], 8), eb3r=br(ws["eb3"], 8),
        eb4r=br(ws["eb4"], 8), eb5r=br(ws["eb5"], 4), n1b1r=br(ws["n1b1"], 4),
        n2b1r=br(ws["n2b1"], 4), n2b2r=ws["n2b2"].reshape(1, 1).astype(f32),
        iota=np.tile(np.arange(WN, dtype=f32), (P, 1)),
        n1b2bc=np.tile(ws["n1b2"].astype(f32) * 64.0, (P, 1)),
    )

    in_maps = []
    for c in range(NC):
        lo, hi, n, rel = core_dat[c]
        ein = np.zeros((384, EPAD), f16)
        ein[0:9, :n] = x[col_s[lo:hi]].T
        ein[9:18, :n] = x[row_s[lo:hi]].T
        ein[18, :n] = ea_s[lo:hi]
        ein[19:275, :n] = u2[g_s[lo:hi]].T
        relw = np.empty((P, NWIN * T), f32)
        for w in range(NWIN):
            relw[:, w * T:(w + 1) * T] = _pack_cols(rel - float(WN) * w, T)
        invc_e = np.ones(EPAD, f32)
        invc_e[:n] = invc_node[row_s[lo:hi]]
        invc_e *= 1.0 / 64.0
        xT2 = np.zeros((P, NPAD), f16)
        xT2[0:9, :NPN] = x[NPN * c:NPN * (c + 1)].T
        u2bT = np.zeros((U_DIM, NPAD), f16)
        u2bT[:, :NPN] = u2[batch[NPN * c:NPN * (c + 1)]].T
        im = dict(shared)
        im.update(ein=ein, relw=relw, invc=_pack_cols(invc_e, T),
                  xT2=xT2, u2bT=u2bT)
        in_maps.append(im)

    # ---------------- build + run ------------------------------------------
    key = (EPAD, tuple(win_tiles))
    if key not in _CACHE:
        _CACHE[key] = _build_module(EPAD, win_tiles, h2_bufs)
    nc, names = _CACHE[key]

    from concourse import bass_utils
    trace = bool(int(os.environ.get("KERNEL_TRACE", "0")))
    if trace:
        try:
            import types
            import antenv
            if not hasattr(antenv, "axon_hooks"):
                mod = types.ModuleType("antenv.axon_hooks")
                mod._hook = None
                mod.set_axon_ntff_profile_hook = lambda h: setattr(mod, "_hook", h)
                mod.get_axon_ntff_profile_hook = lambda: mod._hook
                sys.modules["antenv.axon_hooks"] = mod
                antenv.axon_hooks = mod
                from trn_agent_boot.trn_boot import _ntff_profile_via_ctypes
                mod._hook = _ntff_profile_via_ctypes("/opt/axon/libaxon_pjrt.so")
        except Exception as e:  # profiling is best-effort
            print("ntff hook shim failed:", e)
            trace = False
    real_maps = [{names[k]: v for k, v in im.items()} for im in in_maps]
    res = bass_utils.run_bass_kernel_spmd(
        nc, real_maps, core_ids=list(range(NC)), trace=trace)
    if trace and res.exec_time_ns is not None:
        print(f"HW exec time: {res.exec_time_ns} ns")
        if res.instructions_and_trace:
            print("trace:", res.instructions_and_trace[1])

    out = np.empty(N_NODES, f32)
    for c in range(NC):
        out[NPN * c:NPN * (c + 1)] = res.results[c][names["zout"]][0, :NPN]
    return out
